# revision 2
# baseline (speedup 1.0000x reference)
"""Trainium2 Bass kernel for nn_MeanAligning (VQ codebook mean-aligning loss), v2.

Hybrid stream: most one-hot `code` tiles are DMA-streamed as fp8 (lossless
0/1 recompression), but ~24 of the 64 DoubleRow matmuls get their rhs
SYNTHESIZED on-device from uint16 indices (iota is_eq idx on DVE; Abs+Relu
chain on ACT) — trading idle vector-engine cycles for ~3MB of HBM traffic.

Epilogue is mask-free: loss_sum = sum((cb-mean')^2) - sum_all(cb^2) +
sum_valid(cb^2), with per-channel partial sums fused into the elementwise
ops via accum_out; the final cross-partition reduction happens in the host
combine step (per-core [32,4] partials).
"""

import os
import sys

import numpy as np

for _p in (
    "/opt/trn_rl_repo",
    "/root/.axon_site",
    "/root/.axon_site/_ro/trn_rl_repo",
):
    if os.path.isdir(_p) and _p not in sys.path:
        sys.path.append(_p)

import concourse.bass as bass  # noqa: E402
import concourse.mybir as mybir  # noqa: E402
import concourse.tile as tile  # noqa: E402
from concourse import bacc, bass_utils  # noqa: E402
from concourse.bass import ts  # noqa: E402

F32 = mybir.dt.float32
F16 = mybir.dt.float16
BF16 = mybir.dt.bfloat16
FP8 = mybir.dt.float8e4
U16 = mybir.dt.uint16
AOT = mybir.AluOpType
AF = mybir.ActivationFunctionType

# Problem shapes (hardcoded per contract).
N, H, W, C, K = 16, 32, 32, 32, 4096
NHW = N * H * W            # 16384 positions
NCORES = 8
KS = K // NCORES           # 512 codebook entries per core
P = 128                    # partitions
S = NHW // P               # 128 position-tiles
A = S // 2                 # 64 DoubleRow matmuls (position-pairs)
C1 = C + 1                 # 33 = C + ones column
C1P = 48                   # padded qo row length (DoubleRow step%16==0)

# Stream split (tunable via env for experiments)
N_DVE = int(os.environ.get("MA2_NDVE", "20"))   # DVE-generated matmuls
N_ACT = int(os.environ.get("MA2_NACT", "4"))    # ACT-generated matmuls
N_GEN = N_DVE + N_ACT
N_DMA = A - N_GEN                                # DMA-streamed matmuls
GB = int(os.environ.get("MA2_GB", "8"))          # pos-tiles per DMA batch
NBD = (2 * N_DMA) // GB                          # DMA batches
assert N_DMA * 2 == NBD * GB, "DMA share must be whole batches"
CODE_COLS = GB * KS
OOB = 1024                 # sentinel for out-of-shard indices
N_WARM = int(os.environ.get("MA2_NWARM", "8"))   # HAM warm-up matmuls

# queue assignment for code batches (NBD=11 default):
# tensor-engine queue gets a few mid-stream batches
TENSOR_BATCHES = [int(x) for x in
                  os.environ.get("MA2_TB", "").split(",") if x != ""]
_rest = [t for t in range(NBD) if t not in TENSOR_BATCHES]
# sync ring measured faster (~220B/ns vs ~175): it takes the extra batch
_ns = (len(_rest) + 1) // 2
SYNC_BATCHES = _rest[0::2][:_ns] + ([_rest[-1]] if len(_rest) % 2 == 0 else [])
SYNC_BATCHES = sorted(set(_rest[0::2] + ([_rest[-1]] if len(_rest) % 2 == 0 else [])))
SCALAR_BATCHES = [t for t in _rest if t not in SYNC_BATCHES]
# approximate arrival order across the queues (sync ~2.3us/batch after
# ~12.8 first; scalar ~2.9us/batch after ~11.5)
def _batch_order():
    arr = {}
    for i, t in enumerate(SYNC_BATCHES):
        arr[t] = 12.8 + 2.3 * i
    for i, t in enumerate(SCALAR_BATCHES):
        arr[t] = 11.5 + 2.9 * i
    for i, t in enumerate(TENSOR_BATCHES):
        arr[t] = 15.0 + 2.8 * i
    return sorted(range(NBD), key=lambda t: arr[t])
BATCH_ORDER = _batch_order()

_CACHE: dict = {}


def _schedule():
    """Global matmul order sorted by expected data-ready time.

    DMA batches are spread over three issue queues (sync / scalar /
    tensor); BATCH_ORDER approximates their arrival order.
    """
    sched = []
    gds = [("dve", a) for a in range(N_DVE)]
    gas = [("act", a) for a in range(N_DVE, N_GEN)]

    def take(lst, n):
        for _ in range(min(n, len(lst))):
            sched.append(lst.pop(0))

    take(gds, 4)
    for i, t in enumerate(BATCH_ORDER):
        for b in range(GB // 2):
            sched.append(("dma", t, b))
        if i % 2 == 0:
            take(gds, 1)
        else:
            take(gds, 2)
        if i % 2 == 1:
            take(gas, 1)
    take(gds, 99)
    take(gas, 99)
    assert len(sched) == A, len(sched)
    return sched


def _build_nc():
    nc = bacc.Bacc(
        "TRN2",
        target_bir_lowering=False,
        debug=False,
        enable_asserts=False,
        num_devices=NCORES,
    )

    # idx_t[p, s] = local codebook index of position s*128+p (OOB -> 1024)
    idx_d = nc.dram_tensor("idx_t", [P, S], U16, kind="ExternalInput").ap()
    # qo[p, (a j c)] = [quant | ones | 0pad][(2a+j)*128 + p, c]  (fp8, c=48)
    qo_d = nc.dram_tensor("qo", [P, S * C1P], FP8, kind="ExternalInput").ap()
    # cbt[c, k] = codebook[k_shard_base + k, c]  (fp16)
    cb_d = nc.dram_tensor("cbt", [C, KS], F16, kind="ExternalInput").ap()
    # code_s[t*128+p, g*512+k] = code[pos-tile (2*N_GEN + t*16 + g), k] (fp8)
    code_d = nc.dram_tensor(
        "code_s", [NBD * P, CODE_COLS], FP8, kind="ExternalInput").ap()
    # loss: per-channel sums of (cb - mean')^2 over this core's k-shard
    loss_d = nc.dram_tensor("loss", [C, 1], F32, kind="ExternalOutput").ap()
    warm_d = nc.dram_tensor("warmj", [1, 1], F32, kind="ExternalOutput").ap()

    sched = _schedule()

    with tile.TileContext(nc) as tc:
        with (
            tc.tile_pool(name="consts", bufs=1) as consts,
            tc.tile_pool(name="codep", bufs=1) as codep,
            tc.tile_pool(name="genp", bufs=8) as genp,
            tc.tile_pool(name="actp", bufs=4) as actp,
            tc.tile_pool(name="work", bufs=1) as work,
            tc.tile_pool(name="acc_psum", bufs=1, space="PSUM") as acc_psum,
            tc.tile_pool(name="aux_psum", bufs=2, space="PSUM") as aux_psum,
        ):
            # ---- PE warm-up: one accumulation group of dummy matmuls so
            # HAM reaches K=8/8 during the DMA dead-time (PE idle anyway;
            # without this the first ~12 real MMs run at 1.2 GHz) ----
            wu_sb = consts.tile([P, KS], FP8)
            nc.vector.memset(wu_sb, 1.0)
            wu_ps = aux_psum.tile([C, KS], F32, tag="warm")
            for w in range(N_WARM):
                nc.tensor.matmul(wu_ps, wu_sb[:, 0:C], wu_sb,
                                 start=(w == 0), stop=(w == N_WARM - 1))

            # ---- DMA plan: three issue queues, need-ordered ----
            # Per-queue DMA throughput caps near 180 GB/s; sync and scalar
            # carry the front of the stream, and the tensor engine issues a
            # few mid-stream batches (emitted after the first matmul so PE
            # start is not delayed). Tiny gen-critical transfers (idx, cb,
            # first qo chunk) go first on their rings.
            idx_sb = consts.tile([P, S], U16)
            nc.sync.dma_start(idx_sb, idx_d)
            cb_sb = consts.tile([C, KS], F16)
            nc.scalar.dma_start(cb_sb, cb_d)

            iota16 = consts.tile([P, KS], U16)
            nc.gpsimd.iota(iota16, [[1, KS]], base=0, channel_multiplier=0)

            qo_sb = consts.tile([P, S * C1P], FP8)
            qcuts = [0, N_GEN, min(N_GEN + 16, A), min(N_GEN + 32, A), A]
            qcols = [c * 2 * C1P for c in qcuts]

            ctiles = []
            for t in range(NBD):
                ctile = codep.tile([P, CODE_COLS], FP8, tag=f"code{t}",
                                   name=f"ctile{t}")
                ctiles.append(ctile)

            def code_dma(eng, t):
                eng.dma_start(ctiles[t], code_d[ts(t, P), :])

            def qo_dma(eng, i):
                eng.dma_start(qo_sb[:, qcols[i]:qcols[i + 1]],
                              qo_d[:, qcols[i]:qcols[i + 1]])

            # sync ring: idx, qo_gen, then its code batches + qo_c3
            qo_dma(nc.sync, 0)
            for i, t in enumerate(SYNC_BATCHES):
                code_dma(nc.sync, t)
                if i == 0 and qcols[3] > qcols[2]:
                    qo_dma(nc.sync, 2)
            # scalar ring prefix: cb, first batch, qo_c2; rest interleave
            # with ACT gen pairs via pop_scalar_dma
            if SCALAR_BATCHES:
                code_dma(nc.scalar, SCALAR_BATCHES[0])
            if qcols[2] > qcols[1]:
                qo_dma(nc.scalar, 1)
            scalar_dmas = []
            for t in SCALAR_BATCHES[1:]:
                scalar_dmas.append(("code", t))
            if qcols[4] > qcols[3]:
                scalar_dmas.insert(min(2, len(scalar_dmas)), ("qo", 3))
            emitted = set(SYNC_BATCHES + TENSOR_BATCHES + SCALAR_BATCHES[:1])

            def pop_scalar_dma(n=1):
                for _ in range(min(n, len(scalar_dmas))):
                    it = scalar_dmas.pop(0)
                    if it[0] == "code":
                        code_dma(nc.scalar, it[1])
                        emitted.add(it[1])
                    else:
                        qo_dma(nc.scalar, it[1])

            def ensure_batch_dma(t):
                while t not in emitted:
                    assert scalar_dmas, f"batch {t} dma unaccounted"
                    pop_scalar_dma(1)

            # ---- small consts / index prep (DVE) ----
            idxf = consts.tile([P, S], F32)
            nc.vector.tensor_copy(idxf, idx_sb)
            nidxf = consts.tile([P, S], F32)
            nc.vector.tensor_scalar_mul(nidxf, idxf, -1.0)
            ones1 = consts.tile([1, C], F16)
            nc.vector.memset(ones1, 1.0)
            out4 = consts.tile([C, 1], F32)
            nc.vector.memset(out4, 0.0)

            # ---- main stream: 64 DoubleRow matmuls into acc ----
            acc = acc_psum.tile([C1, KS], F32)  # rows 0..31 num^T, row 32 count
            qo3 = qo_sb.rearrange("p (a j c) -> p a j c", j=2, c=C1P)

            def emit_mm(a, rhs3, first, last):
                nc.tensor.matmul(
                    acc, qo3[:, a, :, 0:C1], rhs3,
                    start=first, stop=last,
                    perf_mode=mybir.MatmulPerfMode.DoubleRow,
                )

            n_act_seen = 0
            for i, item in enumerate(sched):
                first, last = (i == 0), (i == A - 1)
                if i == 1:
                    for t in TENSOR_BATCHES:
                        code_dma(nc.gpsimd, t)
                if item[0] == "dve":
                    a = item[1]
                    gt = genp.tile([P, 2 * KS], FP8, tag="gen")
                    for j in range(2):
                        nc.vector.tensor_scalar(
                            gt[:, j * KS:(j + 1) * KS], iota16,
                            idxf[:, 2 * a + j:2 * a + j + 1], None,
                            AOT.is_equal)
                    emit_mm(a, gt.rearrange("p (j k) -> p j k", j=2),
                            first, last)
                elif item[0] == "act":
                    a = item[1]
                    gt = genp.tile([P, 2 * KS], FP8, tag="gen")
                    for j in range(2):
                        s = 2 * a + j
                        t1 = actp.tile([P, KS], F16, tag="acttmp")
                        nc.scalar.activation(
                            t1, iota16, AF.Abs,
                            bias=nidxf[:, s:s + 1], scale=1.0)
                        nc.scalar.activation(
                            gt[:, j * KS:(j + 1) * KS], t1, AF.Relu,
                            bias=1.0, scale=-1.0)
                    emit_mm(a, gt.rearrange("p (j k) -> p j k", j=2),
                            first, last)
                    n_act_seen += 1
                    pop_scalar_dma(1 if n_act_seen > 1 else 0)
                else:
                    _, t, b = item
                    if b == 0:
                        # descriptors must be emitted before their consumers
                        ensure_batch_dma(t)
                    a = N_GEN + (GB // 2) * t + b
                    ct3 = ctiles[t].rearrange("p (g k) -> p g k", k=KS)
                    emit_mm(a, ct3[:, 2 * b:2 * b + 2, :], first, last)
            pop_scalar_dma(99)

            # ---- epilogue ----
            # ACT (parallel): num copy PSUM->SBUF fp16
            num_sb = work.tile([C, KS], F16)
            nc.scalar.activation(num_sb, acc[0:C, :], AF.Copy)

            # DVE chain
            cntm = work.tile([1, KS], F32)
            nc.vector.tensor_scalar_max(cntm, acc[C:C1, :], 0.5)
            rcp = work.tile([1, KS], F32)
            nc.vector.reciprocal_approx_fast(rcp, cntm)
            rcp16 = work.tile([1, KS], F16)
            nc.vector.tensor_copy(rcp16, rcp)
            # broadcast 1/count across the 32 C partitions via PE (fp16 1-pass)
            bc_ps = aux_psum.tile([C, KS], F32, tag="bc")
            nc.tensor.matmul(bc_ps, ones1, rcp16, start=True, stop=True)
            # mean' = num * rcp_b; invalid k have num=0 so mean'=0 (self-masked)
            mean = work.tile([C, KS], F16)
            nc.vector.tensor_mul(mean, num_sb, bc_ps)
            diff = work.tile([C, KS], F16)
            nc.vector.tensor_sub(diff, cb_sb, mean)
            sqjunk = work.tile([C, KS], F16)
            nc.vector.scalar_tensor_tensor(
                sqjunk, diff, 1.0, diff, AOT.bypass, AOT.mult,
                accum_out=out4)

            nc.sync.dma_start(loss_d, out4)
            if N_WARM:
                # junk copy+DMA keeps the warm-up chain live (idle engines)
                wu_junk = work.tile([1, 1], F32)
                nc.scalar.activation(wu_junk, wu_ps[0:1, 0:1], AF.Copy)
                nc.scalar.dma_start(warm_d, wu_junk)

    nc.compile()
    return nc


def _get_nc():
    if "nc" not in _CACHE:
        _CACHE["nc"] = _build_nc()
    return _CACHE["nc"]


def _make_in_maps(quantized, code, codebook):
    np_fp8 = mybir.dt.np(FP8)

    q2 = np.asarray(quantized, dtype=np.float32).reshape(NHW, C)
    code2 = np.asarray(code, dtype=np.float32).reshape(NHW, K)
    cb = np.asarray(codebook, dtype=np.float32)
    idx = np.argmax(code2, axis=1)  # exact: code is one-hot
    _CACHE["idx"] = idx
    _CACHE["cb"] = cb

    qo = np.zeros((NHW, C1P), np.float32)
    qo[:, 0:C] = q2
    qo[:, C] = 1.0
    qo_kc = np.ascontiguousarray(
        qo.reshape(S, P, C1P).swapaxes(0, 1)
    ).reshape(P, S * C1P).astype(np_fp8)

    in_maps = []
    for j in range(NCORES):
        lo, hi = j * KS, (j + 1) * KS
        idx_loc = idx - lo
        idx_loc = np.where((idx >= lo) & (idx < hi), idx_loc, OOB).astype(np.uint16)
        # idx_t[p, s] = idx_loc[s*128 + p]
        idx_t = np.ascontiguousarray(idx_loc.reshape(S, P).T)

        code8 = code2[:, lo:hi].astype(np_fp8)  # 0/1 values: exact
        # DMA part: pos-tiles 2*N_GEN .. 127 -> [NBD, GB, P, KS] -> [NBD,P,GB,KS]
        pos0 = 2 * N_GEN * P
        code_j = np.ascontiguousarray(
            code8[pos0:].reshape(NBD, GB, P, KS).transpose(0, 2, 1, 3)
        ).reshape(NBD * P, CODE_COLS)
        cbt_j = np.ascontiguousarray(cb[lo:hi].T).astype(np.float16)  # [32,512]
        in_maps.append(
            {"idx_t": idx_t, "qo": qo_kc, "cbt": cbt_j, "code_s": code_j})
    return in_maps


def run(quantized, code, codebook, trace=False, **spmd_kwargs):
    """Run the SPMD kernel; returns (loss_scalar, BassKernelResults)."""
    nc = _get_nc()
    in_maps = _make_in_maps(quantized, code, codebook)
    res = bass_utils.run_bass_kernel_spmd(
        nc, in_maps, core_ids=list(range(NCORES)), trace=trace, **spmd_kwargs
    )
    parts = np.stack(
        [np.asarray(res.results[j]["loss"]).reshape(C) for j in range(NCORES)]
    ).astype(np.float64)
    sq_sum = parts.sum()                # sum (cb - mean')^2 over all k
    # validity bookkeeping from the index histogram (host-side O(K) scalars)
    idx = _CACHE["idx"]
    count = np.bincount(idx, minlength=K)
    valid = count > 0
    cbsq_k = (np.asarray(_CACHE["cb"], np.float64) ** 2).sum(axis=1)  # [K]
    masked = sq_sum - cbsq_k.sum() + cbsq_k[valid].sum()
    nv = float(valid.sum())
    loss = np.float32(masked / (max(nv, 1.0) * C))
    return np.asarray(loss, dtype=np.float32).reshape(()), res


def kernel(quantized, code, codebook):
    loss, _ = run(quantized, code, codebook)
    return loss


# revision 3
# speedup vs baseline: 1.0308x; 1.0308x over previous
"""Trainium2 Bass kernel for nn_MeanAligning (VQ codebook mean-aligning loss), v2.

Hybrid stream: most one-hot `code` tiles are DMA-streamed as fp8 (lossless
0/1 recompression), but ~24 of the 64 DoubleRow matmuls get their rhs
SYNTHESIZED on-device from uint16 indices (iota is_eq idx on DVE; Abs+Relu
chain on ACT) — trading idle vector-engine cycles for ~3MB of HBM traffic.

Epilogue is mask-free: loss_sum = sum((cb-mean')^2) - sum_all(cb^2) +
sum_valid(cb^2), with per-channel partial sums fused into the elementwise
ops via accum_out; the final cross-partition reduction happens in the host
combine step (per-core [32,4] partials).
"""

import os
import sys

import numpy as np

for _p in (
    "/opt/trn_rl_repo",
    "/root/.axon_site",
    "/root/.axon_site/_ro/trn_rl_repo",
):
    if os.path.isdir(_p) and _p not in sys.path:
        sys.path.append(_p)

import concourse.bass as bass  # noqa: E402
import concourse.mybir as mybir  # noqa: E402
import concourse.tile as tile  # noqa: E402
from concourse import bacc, bass_utils  # noqa: E402
from concourse.bass import ts  # noqa: E402

F32 = mybir.dt.float32
F16 = mybir.dt.float16
BF16 = mybir.dt.bfloat16
FP8 = mybir.dt.float8e4
U16 = mybir.dt.uint16
AOT = mybir.AluOpType
AF = mybir.ActivationFunctionType

# Problem shapes (hardcoded per contract).
N, H, W, C, K = 16, 32, 32, 32, 4096
NHW = N * H * W            # 16384 positions
NCORES = 8
KS = K // NCORES           # 512 codebook entries per core
P = 128                    # partitions
S = NHW // P               # 128 position-tiles
A = S // 2                 # 64 DoubleRow matmuls (position-pairs)
C1 = C + 1                 # 33 = C + ones column
C1P = 48                   # padded qo row length (DoubleRow step%16==0)

# Stream split (tunable via env for experiments)
N_DVE = int(os.environ.get("MA2_NDVE", "20"))   # DVE-generated matmuls
N_ACT = int(os.environ.get("MA2_NACT", "4"))    # ACT-generated matmuls
N_GEN = N_DVE + N_ACT
N_DMA = A - N_GEN                                # DMA-streamed matmuls
GB = int(os.environ.get("MA2_GB", "8"))          # pos-tiles per DMA batch
NBD = (2 * N_DMA) // GB                          # DMA batches
assert N_DMA * 2 == NBD * GB, "DMA share must be whole batches"
CODE_COLS = GB * KS
OOB = 1024                 # sentinel for out-of-shard indices
N_WARM = int(os.environ.get("MA2_NWARM", "8"))   # HAM warm-up matmuls

# queue assignment for code batches (NBD=11 default):
# tensor-engine queue gets a few mid-stream batches
TENSOR_BATCHES = [int(x) for x in
                  os.environ.get("MA2_TB", "").split(",") if x != ""]
_rest = [t for t in range(NBD) if t not in TENSOR_BATCHES]
SYNC_BATCHES = _rest[0::2]
SCALAR_BATCHES = _rest[1::2]
# arrival model fitted to measured per-batch DMA completions
def _batch_order():
    arr = {}
    for i, t in enumerate(SYNC_BATCHES):
        arr[t] = 12.9 + 2.55 * i
    for i, t in enumerate(SCALAR_BATCHES):
        arr[t] = 11.9 + 2.85 * i
    for i, t in enumerate(TENSOR_BATCHES):
        arr[t] = 15.0 + 2.8 * i
    return sorted(range(NBD), key=lambda t: arr[t])
BATCH_ORDER = _batch_order()

_CACHE: dict = {}


def _schedule():
    """Global matmul order sorted by expected data-ready time.

    DMA batches are spread over three issue queues (sync / scalar /
    tensor); BATCH_ORDER approximates their arrival order.
    """
    sched = []
    gds = [("dve", a) for a in range(N_DVE)]
    gas = [("act", a) for a in range(N_DVE, N_GEN)]

    def take(lst, n):
        for _ in range(min(n, len(lst))):
            sched.append(lst.pop(0))

    take(gds, 4)
    for i, t in enumerate(BATCH_ORDER):
        for b in range(GB // 2):
            sched.append(("dma", t, b))
        if i % 2 == 0:
            take(gds, 1)
        else:
            take(gds, 2)
        if i % 2 == 1:
            take(gas, 1)
    take(gds, 99)
    take(gas, 99)
    assert len(sched) == A, len(sched)
    return sched


def _build_nc():
    nc = bacc.Bacc(
        "TRN2",
        target_bir_lowering=False,
        debug=False,
        enable_asserts=False,
        num_devices=NCORES,
    )

    # idx_t[p, s] = local codebook index of position s*128+p (OOB -> 1024)
    idx_d = nc.dram_tensor("idx_t", [P, S], U16, kind="ExternalInput").ap()
    # qo[p, (a j c)] = [quant | ones | 0pad][(2a+j)*128 + p, c]  (fp8, c=48)
    qo_d = nc.dram_tensor("qo", [P, S * C1P], FP8, kind="ExternalInput").ap()
    # cbt[c, k] = codebook[k_shard_base + k, c]  (fp16)
    cb_d = nc.dram_tensor("cbt", [C, KS], F16, kind="ExternalInput").ap()
    # code_s[t*128+p, g*512+k] = code[pos-tile (2*N_GEN + t*16 + g), k] (fp8)
    code_d = nc.dram_tensor(
        "code_s", [NBD * P, CODE_COLS], FP8, kind="ExternalInput").ap()
    # loss: per-channel sums of (cb - mean')^2 over this core's k-shard
    loss_d = nc.dram_tensor("loss", [C, 1], F32, kind="ExternalOutput").ap()
    warm_d = nc.dram_tensor("warmj", [1, 1], F32, kind="ExternalOutput").ap()

    sched = _schedule()

    with tile.TileContext(nc) as tc:
        with (
            tc.tile_pool(name="consts", bufs=1) as consts,
            tc.tile_pool(name="codep", bufs=1) as codep,
            tc.tile_pool(name="genp", bufs=8) as genp,
            tc.tile_pool(name="actp", bufs=4) as actp,
            tc.tile_pool(name="work", bufs=1) as work,
            tc.tile_pool(name="acc_psum", bufs=1, space="PSUM") as acc_psum,
            tc.tile_pool(name="aux_psum", bufs=2, space="PSUM") as aux_psum,
        ):
            # ---- PE warm-up: one accumulation group of dummy matmuls so
            # HAM reaches K=8/8 during the DMA dead-time (PE idle anyway;
            # without this the first ~12 real MMs run at 1.2 GHz) ----
            wu_sb = consts.tile([P, KS], FP8)
            nc.vector.memset(wu_sb, 1.0)
            wu_ps = aux_psum.tile([C, KS], F32, tag="warm")
            for w in range(N_WARM):
                nc.tensor.matmul(wu_ps, wu_sb[:, 0:C], wu_sb,
                                 start=(w == 0), stop=(w == N_WARM - 1))

            # ---- DMA plan: three issue queues, need-ordered ----
            # Per-queue DMA throughput caps near 180 GB/s; sync and scalar
            # carry the front of the stream, and the tensor engine issues a
            # few mid-stream batches (emitted after the first matmul so PE
            # start is not delayed). Tiny gen-critical transfers (idx, cb,
            # first qo chunk) go first on their rings.
            idx_sb = consts.tile([P, S], U16)
            nc.sync.dma_start(idx_sb, idx_d)
            cb_sb = consts.tile([C, KS], F16)
            nc.scalar.dma_start(cb_sb, cb_d)

            iota16 = consts.tile([P, KS], U16)
            nc.gpsimd.iota(iota16, [[1, KS]], base=0, channel_multiplier=0)

            qo_sb = consts.tile([P, S * C1P], FP8)
            qcuts = [0, N_GEN, min(N_GEN + 16, A), min(N_GEN + 32, A), A]
            qcols = [c * 2 * C1P for c in qcuts]

            ctiles = []
            for t in range(NBD):
                ctile = codep.tile([P, CODE_COLS], FP8, tag=f"code{t}",
                                   name=f"ctile{t}")
                ctiles.append(ctile)

            def code_dma(eng, t):
                eng.dma_start(ctiles[t], code_d[ts(t, P), :])

            def qo_dma(eng, i):
                eng.dma_start(qo_sb[:, qcols[i]:qcols[i + 1]],
                              qo_d[:, qcols[i]:qcols[i + 1]])

            # sync ring: idx, qo_gen, then its code batches + qo_c3
            qo_dma(nc.sync, 0)
            for i, t in enumerate(SYNC_BATCHES):
                code_dma(nc.sync, t)
                if i == 0 and qcols[3] > qcols[2]:
                    qo_dma(nc.sync, 2)
            # scalar ring prefix: cb, first batch, qo_c2; rest interleave
            # with ACT gen pairs via pop_scalar_dma
            _upfront = SCALAR_BATCHES[:3]
            if _upfront:
                code_dma(nc.scalar, _upfront[0])
            if qcols[2] > qcols[1]:
                qo_dma(nc.scalar, 1)
            for t in _upfront[1:]:
                code_dma(nc.scalar, t)
            scalar_dmas = []
            for t in SCALAR_BATCHES[3:]:
                scalar_dmas.append(("code", t))
            if qcols[4] > qcols[3]:
                scalar_dmas.insert(min(1, len(scalar_dmas)), ("qo", 3))
            emitted = set(SYNC_BATCHES + TENSOR_BATCHES + _upfront)

            def pop_scalar_dma(n=1):
                for _ in range(min(n, len(scalar_dmas))):
                    it = scalar_dmas.pop(0)
                    if it[0] == "code":
                        code_dma(nc.scalar, it[1])
                        emitted.add(it[1])
                    else:
                        qo_dma(nc.scalar, it[1])

            def ensure_batch_dma(t):
                while t not in emitted:
                    assert scalar_dmas, f"batch {t} dma unaccounted"
                    pop_scalar_dma(1)

            # ---- small consts / index prep (DVE) ----
            idxf = consts.tile([P, S], F32)
            nc.vector.tensor_copy(idxf, idx_sb)
            nidxf = consts.tile([P, S], F32)
            nc.vector.tensor_scalar_mul(nidxf, idxf, -1.0)
            ones1 = consts.tile([1, C], F16)
            nc.vector.memset(ones1, 1.0)
            out4 = consts.tile([C, 1], F32)
            nc.vector.memset(out4, 0.0)

            # ---- main stream: 64 DoubleRow matmuls into acc ----
            acc = acc_psum.tile([C1, KS], F32)  # rows 0..31 num^T, row 32 count
            qo3 = qo_sb.rearrange("p (a j c) -> p a j c", j=2, c=C1P)

            def emit_mm(a, rhs3, first, last):
                nc.tensor.matmul(
                    acc, qo3[:, a, :, 0:C1], rhs3,
                    start=first, stop=last,
                    perf_mode=mybir.MatmulPerfMode.DoubleRow,
                )

            n_act_seen = 0
            for i, item in enumerate(sched):
                first, last = (i == 0), (i == A - 1)
                if i == 1:
                    for t in TENSOR_BATCHES:
                        code_dma(nc.gpsimd, t)
                if item[0] == "dve":
                    a = item[1]
                    gt = genp.tile([P, 2 * KS], FP8, tag="gen")
                    for j in range(2):
                        nc.vector.tensor_scalar(
                            gt[:, j * KS:(j + 1) * KS], iota16,
                            idxf[:, 2 * a + j:2 * a + j + 1], None,
                            AOT.is_equal)
                    emit_mm(a, gt.rearrange("p (j k) -> p j k", j=2),
                            first, last)
                elif item[0] == "act":
                    a = item[1]
                    gt = genp.tile([P, 2 * KS], FP8, tag="gen")
                    for j in range(2):
                        s = 2 * a + j
                        t1 = actp.tile([P, KS], F16, tag="acttmp")
                        nc.scalar.activation(
                            t1, iota16, AF.Abs,
                            bias=nidxf[:, s:s + 1], scale=1.0)
                        nc.scalar.activation(
                            gt[:, j * KS:(j + 1) * KS], t1, AF.Relu,
                            bias=1.0, scale=-1.0)
                    emit_mm(a, gt.rearrange("p (j k) -> p j k", j=2),
                            first, last)
                    n_act_seen += 1
                    pop_scalar_dma(1 if n_act_seen > 1 else 0)
                else:
                    _, t, b = item
                    if b == 0:
                        # descriptors must be emitted before their consumers
                        ensure_batch_dma(t)
                    a = N_GEN + (GB // 2) * t + b
                    ct3 = ctiles[t].rearrange("p (g k) -> p g k", k=KS)
                    emit_mm(a, ct3[:, 2 * b:2 * b + 2, :], first, last)
            pop_scalar_dma(99)

            # ---- epilogue ----
            # ACT (parallel): num copy PSUM->SBUF fp16
            num_sb = work.tile([C, KS], F16)
            nc.scalar.activation(num_sb, acc[0:C, :], AF.Copy)

            # DVE chain
            cntm = work.tile([1, KS], F32)
            nc.vector.tensor_scalar_max(cntm, acc[C:C1, :], 0.5)
            rcp = work.tile([1, KS], F32)
            nc.vector.reciprocal_approx_fast(rcp, cntm)
            rcp16 = work.tile([1, KS], F16)
            nc.vector.tensor_copy(rcp16, rcp)
            # broadcast 1/count across the 32 C partitions via PE (fp16 1-pass)
            bc_ps = aux_psum.tile([C, KS], F32, tag="bc")
            nc.tensor.matmul(bc_ps, ones1, rcp16, start=True, stop=True)
            # mean' = num * rcp_b; invalid k have num=0 so mean'=0 (self-masked)
            mean = work.tile([C, KS], F16)
            nc.vector.tensor_mul(mean, num_sb, bc_ps)
            diff = work.tile([C, KS], F16)
            nc.vector.tensor_sub(diff, cb_sb, mean)
            sqjunk = work.tile([C, KS], F16)
            nc.vector.scalar_tensor_tensor(
                sqjunk, diff, 1.0, diff, AOT.bypass, AOT.mult,
                accum_out=out4)

            nc.sync.dma_start(loss_d, out4)
            if N_WARM:
                # junk copy+DMA keeps the warm-up chain live (idle engines)
                wu_junk = work.tile([1, 1], F32)
                nc.scalar.activation(wu_junk, wu_ps[0:1, 0:1], AF.Copy)
                nc.scalar.dma_start(warm_d, wu_junk)

    nc.compile()
    return nc


def _get_nc():
    if "nc" not in _CACHE:
        _CACHE["nc"] = _build_nc()
    return _CACHE["nc"]


def _make_in_maps(quantized, code, codebook):
    np_fp8 = mybir.dt.np(FP8)

    q2 = np.asarray(quantized, dtype=np.float32).reshape(NHW, C)
    code2 = np.asarray(code, dtype=np.float32).reshape(NHW, K)
    cb = np.asarray(codebook, dtype=np.float32)
    idx = np.argmax(code2, axis=1)  # exact: code is one-hot
    _CACHE["idx"] = idx
    _CACHE["cb"] = cb

    qo = np.zeros((NHW, C1P), np.float32)
    qo[:, 0:C] = q2
    qo[:, C] = 1.0
    qo_kc = np.ascontiguousarray(
        qo.reshape(S, P, C1P).swapaxes(0, 1)
    ).reshape(P, S * C1P).astype(np_fp8)

    in_maps = []
    for j in range(NCORES):
        lo, hi = j * KS, (j + 1) * KS
        idx_loc = idx - lo
        idx_loc = np.where((idx >= lo) & (idx < hi), idx_loc, OOB).astype(np.uint16)
        # idx_t[p, s] = idx_loc[s*128 + p]
        idx_t = np.ascontiguousarray(idx_loc.reshape(S, P).T)

        code8 = code2[:, lo:hi].astype(np_fp8)  # 0/1 values: exact
        # DMA part: pos-tiles 2*N_GEN .. 127 -> [NBD, GB, P, KS] -> [NBD,P,GB,KS]
        pos0 = 2 * N_GEN * P
        code_j = np.ascontiguousarray(
            code8[pos0:].reshape(NBD, GB, P, KS).transpose(0, 2, 1, 3)
        ).reshape(NBD * P, CODE_COLS)
        cbt_j = np.ascontiguousarray(cb[lo:hi].T).astype(np.float16)  # [32,512]
        in_maps.append(
            {"idx_t": idx_t, "qo": qo_kc, "cbt": cbt_j, "code_s": code_j})
    return in_maps


def run(quantized, code, codebook, trace=False, **spmd_kwargs):
    """Run the SPMD kernel; returns (loss_scalar, BassKernelResults)."""
    nc = _get_nc()
    in_maps = _make_in_maps(quantized, code, codebook)
    res = bass_utils.run_bass_kernel_spmd(
        nc, in_maps, core_ids=list(range(NCORES)), trace=trace, **spmd_kwargs
    )
    parts = np.stack(
        [np.asarray(res.results[j]["loss"]).reshape(C) for j in range(NCORES)]
    ).astype(np.float64)
    sq_sum = parts.sum()                # sum (cb - mean')^2 over all k
    # validity bookkeeping from the index histogram (host-side O(K) scalars)
    idx = _CACHE["idx"]
    count = np.bincount(idx, minlength=K)
    valid = count > 0
    cbsq_k = (np.asarray(_CACHE["cb"], np.float64) ** 2).sum(axis=1)  # [K]
    masked = sq_sum - cbsq_k.sum() + cbsq_k[valid].sum()
    nv = float(valid.sum())
    loss = np.float32(masked / (max(nv, 1.0) * C))
    return np.asarray(loss, dtype=np.float32).reshape(()), res


def kernel(quantized, code, codebook):
    loss, _ = run(quantized, code, codebook)
    return loss
